# revision 20
# baseline (speedup 1.0000x reference)
"""Masked grouped Conv1D (CustomMaskedConv1D) Trainium2 Bass kernel.

Problem (reference semantics):
  inputs    [B=4, L=4096, C=1024] f32
  positions [B=4, L=4096] i32 (sorted)
  kernel    [G=16, OPG=64, IPG=64, K=5] f32
  out[b,l,g,o] = sum_k mask[b,l,k] * sum_i x_pad[b, l+k-2, g*64+i] * W[g,o,i,k]
  mask[b,l,k] = (pos_pad[b, l+k-2] == pos[b,l] + k - 2)

Strategy: data-parallel over (batch x half-sequence) -> 8 shards of 2048 rows
(+2 halo rows each side). Host does lossless layout transforms only
(slicing, zero-pad, transpose, weight repacking, integer index arithmetic on
the int positions); all dtype casts and all arithmetic on tensor data run on
device.

Mask trick: with e[m] = pos[m] - m (int, sentinel e = -1 - m for padded rows
reproducing the reference's -1-sentinel semantics), the mask for tap ki at
padded row m is MK_ki[m] = (e[m] == e[m + (2-ki)]), so masks are 4 int16
is_equal ops on DVE over a replicated e panel (no per-tap subtract needed).

Device pipeline per core:
  - e panel [128, NP+4] i16 (0.53MB) DMA'd first; 4 DVE is_equal masks
  - x^T shard f32 [1024, 2052] DMA'd in 8 chunks, cast to bf16 on ACT
  - compact weights [128, K*NCC*64] f32 (1.31MB instead of 2.6MB block-diag)
    expanded on device: gpsimd memset (ki=0 block first) + strided ACT casts
  - production (DVE 2x): ym_k = xT * MK_k, 4 off-center taps
  - conv (PE): per channel-pair cc and tap k: psum[128, 512] +=
    Wbd[k,cc][128,128]^T @ ym_k[:, n+k:n+k+512] (center tap reads xT directly)
  - psum -> SBUF [128, 2048] staging on ACT, out DMA'd in 2 halves per cc
Host gathers/transposes shards into [4, 4096, 16, 64].
"""

import os

import numpy as np

import concourse.bass as bass
import concourse.mybir as mybir
import concourse.tile as tile
from concourse import bacc
from concourse.bass_utils import run_bass_kernel_spmd

B, L, C = 4, 4096, 1024
G, OPG, IPG, K = 16, 64, 64, 5
HALO = K // 2  # 2

NCORES = 8
NR = (B * L) // NCORES  # 2048 output rows per core
NP = NR + 2 * HALO  # 2052 padded rows per core
NE = NP + 4  # 2056 e-vector length (rows m in [-2, NP+2))
NCC = C // 128  # 8 channel chunks == group pairs
NNB = NR // 512  # 4 n-blocks of 512
TAPS_OFF = (0, 1, 3, 4)  # off-center taps (center tap k=2 has mask==1)
KI_ORDER = (2, 0, 1, 3, 4)  # PE issue order: unmasked center tap first

# cache the compiled Bass program + results of the last run
_NC = None
LAST_RESULTS = None


def _build():
    nc = bacc.Bacc(
        "TRN2", target_bir_lowering=False, debug=False, num_devices=NCORES
    )
    bf16 = mybir.dt.bfloat16
    f32 = mybir.dt.float32
    i16 = mybir.dt.int16

    xt_dram = nc.dram_tensor("xt", [C, NP], f32, kind="ExternalInput")
    # eb[p, j] = e(m=j-2) = ps(m) - m, replicated over 128 partitions
    # (pad rows: -1 - m)
    eb_dram = nc.dram_tensor("eb", [128, NE], i16, kind="ExternalInput")
    # compact weights [128 ch, K*NCC*64]: per (ki,cc) 64-col block, rows 0:64
    # hold lhsT of group 2cc ([i,o]), rows 64:128 group 2cc+1
    w_dram = nc.dram_tensor("w", [128, K * NCC * 64], f32, kind="ExternalInput")
    out_dram = nc.dram_tensor("out", [C, NR], f32, kind="ExternalOutput")

    with tile.TileContext(nc) as tc:
        with (
            tc.tile_pool(name="persist", bufs=1) as pers,
            tc.tile_pool(name="setup", bufs=1) as setup,
            tc.tile_pool(name="stage", bufs=6) as stage,
            tc.tile_pool(name="ym", bufs=2) as ymp,
            tc.tile_pool(name="osb", bufs=2) as osb,
            tc.tile_pool(name="psum", bufs=2, space="PSUM") as pp,
        ):
            # ---- DMA stream: x0 first, then e panel, early w blocks.. ----
            # w is laid out in PE-issue order (ki = 2,0,1,3,4) so the first
            # two tap-blocks arrive in one early DMA
            # DMA completion time = cumulative stream bytes / bandwidth, so
            # the issue order IS the schedule. First-matmul chain needs
            # w_early then x0a; masks need ebc before the productions.
            XH = 1028  # first-chunk split point (even)
            w32 = setup.tile([128, K * NCC * 64], f32, tag="w32")
            W_EARLY = 2 * NCC * 64  # ki=2 and ki=0 blocks
            nc.sync.dma_start(w32[:, 0:W_EARLY], w_dram[:, 0:W_EARLY])
            x32_0 = stage.tile([128, NP], f32, tag="x32", name="x32_0")
            nc.sync.dma_start(x32_0[:, 0:XH], xt_dram[0:128, 0:XH])
            ebc = setup.tile([128, NE], i16, tag="ebc")
            nc.sync.dma_start(ebc[:], eb_dram[:])
            nc.sync.dma_start(x32_0[:, XH:], xt_dram[0:128, XH:])

            # gpsimd (otherwise idle): zero the block-diag weight tile,
            # early tap-blocks first so the first matmuls aren't gated
            w_sb = pers.tile([128, K * NCC * 128], bf16, tag="w")
            nc.gpsimd.memset(w_sb[:, 0 : 2 * NCC * 128], 0.0)
            nc.gpsimd.memset(w_sb[:, 2 * NCC * 128 :], 0.0)

            # Two wide masks cover all four taps via shifted reads:
            #   T0e[j] = (e[j-2] == e[j])    -> MK_0[m] = T0e[m+2],
            #                                   MK_4[m] = T0e[m]
            #   T1e[j] = (e[j-2] == e[j-1])  -> MK_1[m] = T1e[m+2],
            #                                   MK_3[m] = T1e[m+1]
            NPW = NP + 2
            t0e = pers.tile([128, NPW], bf16, tag="t0e")
            nc.vector.tensor_tensor(
                out=t0e[:], in0=ebc[:, 0:NPW], in1=ebc[:, 2 : 2 + NPW],
                op=mybir.AluOpType.is_equal,
            )
            t1e = pers.tile([128, NPW], bf16, tag="t1e")
            nc.vector.tensor_tensor(
                out=t1e[:], in0=ebc[:, 0:NPW], in1=ebc[:, 1 : 1 + NPW],
                op=mybir.AluOpType.is_equal,
            )
            msk = {
                0: t0e[:, 2 : 2 + NP],
                4: t0e[:, 0:NP],
                1: t1e[:, 2 : 2 + NP],
                3: t1e[:, 1 : 1 + NP],
            }

            # ---- x chunk casts + block-diag weight expansion on ACT ----
            # ACT order: x0 cast, ki=0 w-block casts, x1 cast, w-rest casts;
            # remaining x casts are interleaved with the main loop below.
            xts = []
            x32s = [x32_0]
            for cc in range(1, NCC):
                x32 = stage.tile([128, NP], f32, tag="x32", name=f"x32_{cc}")
                nc.sync.dma_start(x32[:], xt_dram[cc * 128 : (cc + 1) * 128, :])
                x32s.append(x32)
                if cc == 1:  # remaining w blocks after x1
                    nc.sync.dma_start(
                        w32[:, W_EARLY:], w_dram[:, W_EARLY:]
                    )
            for cc in range(NCC):
                xts.append(
                    pers.tile([128, NP], bf16, tag=f"xt{cc}", name=f"xt{cc}")
                )

            def cast_x(cc):
                if cc == 0:  # halves: first center-tap matmuls gate on 0:XH
                    nc.scalar.copy(xts[0][:, 0:XH], x32s[0][:, 0:XH])
                    nc.scalar.copy(xts[0][:, XH:], x32s[0][:, XH:])
                else:
                    nc.scalar.copy(xts[cc][:], x32s[cc][:])

            def expand_w(lo, hi, c0, c1):
                # rows [lo:hi): cast 64-col compact blocks [c0:c1) into the
                # [lo:lo+64)-diagonal half of the 128-col block-diag layout
                dst = w_sb[lo:hi, c0 * 128 : c1 * 128].rearrange(
                    "p (b c) -> p b c", c=128
                )[:, :, lo : lo + 64]
                src = w32[lo:hi, c0 * 64 : c1 * 64].rearrange(
                    "p (b c) -> p b c", c=64
                )
                nc.scalar.copy(dst, src)

            expand_w(0, 64, 0, 2 * NCC)  # ki=2,0 blocks (w_early lands first)
            expand_w(64, 128, 0, 2 * NCC)
            cast_x(0)
            cast_x(1)
            expand_w(0, 64, 2 * NCC, K * NCC)  # ki=1,3,4 blocks
            expand_w(64, 128, 2 * NCC, K * NCC)

            # ---- main loop over channel chunks (= group pairs) ----
            for cc in range(NCC):
                if cc + 2 < NCC:
                    cast_x(cc + 2)

                # production on DVE (bf16 full-width stride-1 => 2x mode)
                ym = {}
                for k in TAPS_OFF:
                    y = ymp.tile([128, NP], bf16, tag=f"ym{k}", name=f"ym{k}_{cc}")
                    nc.vector.tensor_tensor(
                        out=y[:], in0=xts[cc][:], in1=msk[k],
                        op=mybir.AluOpType.mult,
                    )
                    ym[k] = y

                # conv: k outer (weight reuse), n-block inner (psum accumulate)
                psums = []
                for nb in range(NNB):
                    acc = pp.tile([128, 512], f32, tag=f"acc{nb}", name=f"acc{nb}")
                    psums.append(acc)
                # center (unmasked) tap first: its matmuls need only the x
                # cast + early w blocks, not the masks/productions
                for oi, ki in enumerate(KI_ORDER):
                    wcol = (oi * NCC + cc) * 128
                    lhsT = w_sb[:, wcol : wcol + 128]
                    for nb in range(NNB):
                        n0 = nb * 512
                        if ki == 2:
                            rhs = xts[cc][:, n0 + 2 : n0 + 2 + 512]
                        else:
                            rhs = ym[ki][:, n0 + ki : n0 + ki + 512]
                        nc.tensor.matmul(
                            psums[nb][:], lhsT, rhs,
                            start=(oi == 0), stop=(oi == K - 1),
                        )

                # psum -> SBUF staging on ACT; out DMA'd in 2 halves per cc
                # (quarters for the last cc to shorten the drain tail)
                o_sb = osb.tile([128, NR], f32, tag="osb")
                per_dma = 1 if cc == NCC - 1 else 2
                for nb in range(NNB):
                    nc.scalar.copy(o_sb[:, nb * 512 : (nb + 1) * 512], psums[nb][:])
                    if nb % per_dma == per_dma - 1:
                        h0, h1 = (nb + 1 - per_dma) * 512, (nb + 1) * 512
                        nc.sync.dma_start(
                            out_dram[cc * 128 : (cc + 1) * 128, h0:h1],
                            o_sb[:, h0:h1],
                        )

    nc.compile()
    return nc


def _get_nc():
    global _NC
    if _NC is None:
        _NC = _build()
    return _NC


def _shard_inputs(inputs, positions, kernel):
    """Host-side lossless transforms: slice+pad shards, transpose x, compact
    weight packing, integer e-vector (pos[m]-m with -1-m pad sentinel)."""
    in_maps = []
    # compact weights [128 ch, K*NCC*64], tap blocks in PE issue order
    w_cmp = np.empty((128, K * NCC * 64), dtype=np.float32)
    for oi, ki in enumerate(KI_ORDER):
        for cc in range(NCC):
            col = (oi * NCC + cc) * 64
            # lhsT[i, o] = W[g, o, i, ki]
            w_cmp[0:64, col : col + 64] = kernel[2 * cc, :, :, ki].T
            w_cmp[64:128, col : col + 64] = kernel[2 * cc + 1, :, :, ki].T

    half = L // 2  # 2048
    for core in range(NCORES):
        b, h = divmod(core, 2)
        l0 = h * half
        xs = np.zeros((NP, C), dtype=np.float32)
        ps = np.full((NE,), -1, dtype=np.int64)  # col j <-> row m = j-2
        lo, hi = l0 - HALO, l0 + half + HALO
        src_lo, src_hi = max(lo, 0), min(hi, L)
        dst_lo = src_lo - lo
        xs[dst_lo : dst_lo + (src_hi - src_lo)] = inputs[b, src_lo:src_hi]
        ps[2 + dst_lo : 2 + dst_lo + (src_hi - src_lo)] = positions[
            b, src_lo:src_hi
        ]
        # e(m) = ps(m) - m; pad rows (ps=-1) give e = -1 - m, which
        # reproduces the reference's -1-sentinel compare semantics
        e0 = (ps - (np.arange(NE, dtype=np.int64) - 2)).astype(np.int16)
        eb = np.ascontiguousarray(np.broadcast_to(e0, (128, NE)))
        xt = np.ascontiguousarray(xs.T)  # [C, NP]
        in_maps.append({"xt": xt, "eb": eb, "w": w_cmp})
    return in_maps


def kernel(inputs, positions, kernel):
    global LAST_RESULTS
    inputs = np.asarray(inputs, dtype=np.float32)
    positions = np.asarray(positions, dtype=np.int32)
    kernel = np.asarray(kernel, dtype=np.float32)

    nc = _get_nc()
    in_maps = _shard_inputs(inputs, positions, kernel)
    res = run_bass_kernel_spmd(
        nc,
        in_maps,
        core_ids=list(range(NCORES)),
        trace=bool(os.environ.get("BASS_TRACE")),
    )
    LAST_RESULTS = res

    out = np.empty((B, L, G, OPG), dtype=np.float32)
    half = L // 2
    for core in range(NCORES):
        b, h = divmod(core, 2)
        l0 = h * half
        # device output is out^T [C=1024 (g*64+o), NR]
        ot = res.results[core]["out"]
        out[b, l0 : l0 + half] = ot.T.reshape(half, G, OPG)
    return out


# revision 22
# speedup vs baseline: 1.0197x; 1.0197x over previous
"""Masked grouped Conv1D (CustomMaskedConv1D) Trainium2 Bass kernel.

Problem (reference semantics):
  inputs    [B=4, L=4096, C=1024] f32
  positions [B=4, L=4096] i32 (sorted)
  kernel    [G=16, OPG=64, IPG=64, K=5] f32
  out[b,l,g,o] = sum_k mask[b,l,k] * sum_i x_pad[b, l+k-2, g*64+i] * W[g,o,i,k]
  mask[b,l,k] = (pos_pad[b, l+k-2] == pos[b,l] + k - 2)

Strategy: data-parallel over (batch x half-sequence) -> 8 shards of 2048 rows
(+2 halo rows each side). Host does lossless layout transforms only
(slicing, zero-pad, transpose, weight repacking, integer index arithmetic on
the int positions); all dtype casts and all arithmetic on tensor data run on
device.

Mask trick: with e[m] = pos[m] - m (int, sentinel e = -1 - m for padded rows
reproducing the reference's -1-sentinel semantics), the mask for tap ki at
padded row m is MK_ki[m] = (e[m] == e[m + (2-ki)]), so masks are 4 int16
is_equal ops on DVE over a replicated e panel (no per-tap subtract needed).

Device pipeline per core:
  - e panel [128, NP+4] i16 (0.53MB) DMA'd first; 4 DVE is_equal masks
  - x^T shard f32 [1024, 2052] DMA'd in 8 chunks, cast to bf16 on ACT
  - compact weights [128, K*NCC*64] f32 (1.31MB instead of 2.6MB block-diag)
    expanded on device: gpsimd memset (ki=0 block first) + strided ACT casts
  - production (DVE 2x): ym_k = xT * MK_k, 4 off-center taps
  - conv (PE): per channel-pair cc and tap k: psum[128, 512] +=
    Wbd[k,cc][128,128]^T @ ym_k[:, n+k:n+k+512] (center tap reads xT directly)
  - psum -> SBUF [128, 2048] staging on ACT, out DMA'd in 2 halves per cc
Host gathers/transposes shards into [4, 4096, 16, 64].
"""

import os

import numpy as np

import concourse.bass as bass
import concourse.mybir as mybir
import concourse.tile as tile
from concourse import bacc
from concourse.bass_utils import run_bass_kernel_spmd

B, L, C = 4, 4096, 1024
G, OPG, IPG, K = 16, 64, 64, 5
HALO = K // 2  # 2

NCORES = 8
NR = (B * L) // NCORES  # 2048 output rows per core
NP = NR + 2 * HALO  # 2052 padded rows per core
NE = NP + 4  # 2056 e-vector length (rows m in [-2, NP+2))
NCC = C // 128  # 8 channel chunks == group pairs
NNB = NR // 512  # 4 n-blocks of 512
TAPS_OFF = (0, 1, 3, 4)  # off-center taps (center tap k=2 has mask==1)
KI_ORDER = (2, 0, 1, 3, 4)  # PE issue order: unmasked center tap first

# cache the compiled Bass program + results of the last run
_NC = None
LAST_RESULTS = None


def _build():
    nc = bacc.Bacc(
        "TRN2", target_bir_lowering=False, debug=False, num_devices=NCORES
    )
    bf16 = mybir.dt.bfloat16
    f32 = mybir.dt.float32
    i16 = mybir.dt.int16

    xt_dram = nc.dram_tensor("xt", [C, NP], f32, kind="ExternalInput")
    # eb[p, j] = e(m=j-2) = ps(m) - m, replicated over 128 partitions
    # (pad rows: -1 - m)
    eb_dram = nc.dram_tensor("eb", [128, NE], i16, kind="ExternalInput")
    # compact weights [128 ch, K*NCC*64]: per (ki,cc) 64-col block, rows 0:64
    # hold lhsT of group 2cc ([i,o]), rows 64:128 group 2cc+1
    w_dram = nc.dram_tensor("w", [128, K * NCC * 64], f32, kind="ExternalInput")
    out_dram = nc.dram_tensor("out", [C, NR], f32, kind="ExternalOutput")

    with tile.TileContext(nc) as tc:
        with (
            tc.tile_pool(name="persist", bufs=1) as pers,
            tc.tile_pool(name="setup", bufs=1) as setup,
            tc.tile_pool(name="stage", bufs=6) as stage,
            tc.tile_pool(name="ym", bufs=2) as ymp,
            tc.tile_pool(name="osb", bufs=2) as osb,
            tc.tile_pool(name="psum", bufs=2, space="PSUM") as pp,
        ):
            # ---- DMA stream: x0 first, then e panel, early w blocks.. ----
            # w is laid out in PE-issue order (ki = 2,0,1,3,4) so the first
            # two tap-blocks arrive in one early DMA
            # DMA completion time = cumulative stream bytes / bandwidth, so
            # the issue order IS the schedule. First-matmul chain needs
            # w_early then x0a; masks need ebc before the productions.
            XH = 1028  # first-chunk split point (even)
            w32 = setup.tile([128, K * NCC * 64], f32, tag="w32")
            W_EARLY = 2 * NCC * 64  # ki=2 and ki=0 blocks
            nc.sync.dma_start(w32[:, 0:W_EARLY], w_dram[:, 0:W_EARLY])
            x32_0 = stage.tile([128, NP], f32, tag="x32", name="x32_0")
            nc.sync.dma_start(x32_0[:, 0:XH], xt_dram[0:128, 0:XH])
            ebc = setup.tile([128, NE], i16, tag="ebc")
            nc.sync.dma_start(ebc[:], eb_dram[:])
            nc.sync.dma_start(x32_0[:, XH:], xt_dram[0:128, XH:])

            # gpsimd (otherwise idle): zero the block-diag weight tile,
            # early tap-blocks first so the first matmuls aren't gated
            w_sb = pers.tile([128, K * NCC * 128], bf16, tag="w")
            nc.gpsimd.memset(w_sb[:, 0 : 2 * NCC * 128], 0.0)
            nc.gpsimd.memset(w_sb[:, 2 * NCC * 128 :], 0.0)

            # Two wide masks cover all four taps via shifted reads:
            #   T0e[j] = (e[j-2] == e[j])    -> MK_0[m] = T0e[m+2],
            #                                   MK_4[m] = T0e[m]
            #   T1e[j] = (e[j-2] == e[j-1])  -> MK_1[m] = T1e[m+2],
            #                                   MK_3[m] = T1e[m+1]
            NPW = NP + 2
            t0e = pers.tile([128, NPW], bf16, tag="t0e")
            nc.vector.tensor_tensor(
                out=t0e[:], in0=ebc[:, 0:NPW], in1=ebc[:, 2 : 2 + NPW],
                op=mybir.AluOpType.is_equal,
            )
            t1e = pers.tile([128, NPW], bf16, tag="t1e")
            nc.vector.tensor_tensor(
                out=t1e[:], in0=ebc[:, 0:NPW], in1=ebc[:, 1 : 1 + NPW],
                op=mybir.AluOpType.is_equal,
            )
            msk = {
                0: t0e[:, 2 : 2 + NP],
                4: t0e[:, 0:NP],
                1: t1e[:, 2 : 2 + NP],
                3: t1e[:, 1 : 1 + NP],
            }

            # ---- x chunk casts + block-diag weight expansion on ACT ----
            # ACT order: x0 cast, ki=0 w-block casts, x1 cast, w-rest casts;
            # remaining x casts are interleaved with the main loop below.
            xts = []
            x32s = [x32_0]
            for cc in range(1, NCC):
                x32 = stage.tile([128, NP], f32, tag="x32", name=f"x32_{cc}")
                nc.sync.dma_start(x32[:], xt_dram[cc * 128 : (cc + 1) * 128, :])
                x32s.append(x32)
                if cc == 1:  # remaining w blocks after x1
                    nc.sync.dma_start(
                        w32[:, W_EARLY:], w_dram[:, W_EARLY:]
                    )
            for cc in range(NCC):
                xts.append(
                    pers.tile([128, NP], bf16, tag=f"xt{cc}", name=f"xt{cc}")
                )

            def cast_x(cc):
                if cc == 0:  # halves: first center-tap matmuls gate on 0:XH
                    nc.scalar.copy(xts[0][:, 0:XH], x32s[0][:, 0:XH])
                    nc.scalar.copy(xts[0][:, XH:], x32s[0][:, XH:])
                else:
                    nc.scalar.copy(xts[cc][:], x32s[cc][:])

            def expand_w(lo, hi, c0, c1):
                # rows [lo:hi): cast 64-col compact blocks [c0:c1) into the
                # [lo:lo+64)-diagonal half of the 128-col block-diag layout
                dst = w_sb[lo:hi, c0 * 128 : c1 * 128].rearrange(
                    "p (b c) -> p b c", c=128
                )[:, :, lo : lo + 64]
                src = w32[lo:hi, c0 * 64 : c1 * 64].rearrange(
                    "p (b c) -> p b c", c=64
                )
                nc.scalar.copy(dst, src)

            expand_w(0, 64, 0, 2 * NCC)  # ki=2,0 blocks (w_early lands first)
            expand_w(64, 128, 0, 2 * NCC)
            cast_x(0)
            cast_x(1)
            expand_w(0, 64, 2 * NCC, K * NCC)  # ki=1,3,4 blocks
            expand_w(64, 128, 2 * NCC, K * NCC)

            # ---- main loop over channel chunks (= group pairs) ----
            for cc in range(NCC):
                if cc + 2 < NCC:
                    cast_x(cc + 2)

                # production on DVE (bf16 full-width stride-1 => 2x mode)
                ym = {}
                for k in TAPS_OFF:
                    y = ymp.tile([128, NP], bf16, tag=f"ym{k}", name=f"ym{k}_{cc}")
                    nc.vector.tensor_tensor(
                        out=y[:], in0=xts[cc][:], in1=msk[k],
                        op=mybir.AluOpType.mult,
                    )
                    ym[k] = y

                # conv: k outer (weight reuse), n-block inner (psum accumulate)
                # two 2-bank psum tiles per cc; the psum->SBUF copies then
                # move 1024 cols each (half the ACT ops and semaphores)
                accs = [
                    pp.tile([128, 1024], f32, tag=f"acc{h}", name=f"acc{h}_{cc}")
                    for h in range(2)
                ]
                psums = [
                    accs[nb // 2][:, (nb % 2) * 512 : (nb % 2 + 1) * 512]
                    for nb in range(NNB)
                ]
                # center (unmasked) tap first: its matmuls need only the x
                # cast + early w blocks, not the masks/productions
                for oi, ki in enumerate(KI_ORDER):
                    wcol = (oi * NCC + cc) * 128
                    lhsT = w_sb[:, wcol : wcol + 128]
                    for nb in range(NNB):
                        n0 = nb * 512
                        if ki == 2:
                            rhs = xts[cc][:, n0 + 2 : n0 + 2 + 512]
                        else:
                            rhs = ym[ki][:, n0 + ki : n0 + ki + 512]
                        nc.tensor.matmul(
                            psums[nb], lhsT, rhs,
                            start=(oi == 0), stop=(oi == K - 1),
                        )

                # psum -> SBUF staging on ACT; out DMA'd in 2 halves per cc
                o_sb = osb.tile([128, NR], f32, tag="osb")
                for h in range(2):
                    h0, h1 = h * 1024, (h + 1) * 1024
                    nc.scalar.copy(o_sb[:, h0:h1], accs[h][:])
                    nc.sync.dma_start(
                        out_dram[cc * 128 : (cc + 1) * 128, h0:h1],
                        o_sb[:, h0:h1],
                    )

    nc.compile()
    return nc


def _get_nc():
    global _NC
    if _NC is None:
        _NC = _build()
    return _NC


def _shard_inputs(inputs, positions, kernel):
    """Host-side lossless transforms: slice+pad shards, transpose x, compact
    weight packing, integer e-vector (pos[m]-m with -1-m pad sentinel)."""
    in_maps = []
    # compact weights [128 ch, K*NCC*64], tap blocks in PE issue order
    w_cmp = np.empty((128, K * NCC * 64), dtype=np.float32)
    for oi, ki in enumerate(KI_ORDER):
        for cc in range(NCC):
            col = (oi * NCC + cc) * 64
            # lhsT[i, o] = W[g, o, i, ki]
            w_cmp[0:64, col : col + 64] = kernel[2 * cc, :, :, ki].T
            w_cmp[64:128, col : col + 64] = kernel[2 * cc + 1, :, :, ki].T

    half = L // 2  # 2048
    for core in range(NCORES):
        b, h = divmod(core, 2)
        l0 = h * half
        xs = np.zeros((NP, C), dtype=np.float32)
        ps = np.full((NE,), -1, dtype=np.int64)  # col j <-> row m = j-2
        lo, hi = l0 - HALO, l0 + half + HALO
        src_lo, src_hi = max(lo, 0), min(hi, L)
        dst_lo = src_lo - lo
        xs[dst_lo : dst_lo + (src_hi - src_lo)] = inputs[b, src_lo:src_hi]
        ps[2 + dst_lo : 2 + dst_lo + (src_hi - src_lo)] = positions[
            b, src_lo:src_hi
        ]
        # e(m) = ps(m) - m; pad rows (ps=-1) give e = -1 - m, which
        # reproduces the reference's -1-sentinel compare semantics
        e0 = (ps - (np.arange(NE, dtype=np.int64) - 2)).astype(np.int16)
        eb = np.ascontiguousarray(np.broadcast_to(e0, (128, NE)))
        xt = np.ascontiguousarray(xs.T)  # [C, NP]
        in_maps.append({"xt": xt, "eb": eb, "w": w_cmp})
    return in_maps


def kernel(inputs, positions, kernel):
    global LAST_RESULTS
    inputs = np.asarray(inputs, dtype=np.float32)
    positions = np.asarray(positions, dtype=np.int32)
    kernel = np.asarray(kernel, dtype=np.float32)

    nc = _get_nc()
    in_maps = _shard_inputs(inputs, positions, kernel)
    res = run_bass_kernel_spmd(
        nc,
        in_maps,
        core_ids=list(range(NCORES)),
        trace=bool(os.environ.get("BASS_TRACE")),
    )
    LAST_RESULTS = res

    out = np.empty((B, L, G, OPG), dtype=np.float32)
    half = L // 2
    for core in range(NCORES):
        b, h = divmod(core, 2)
        l0 = h * half
        # device output is out^T [C=1024 (g*64+o), NR]
        ot = res.results[core]["out"]
        out[b, l0 : l0 + half] = ot.T.reshape(half, G, OPG)
    return out
